# revision 18
# baseline (speedup 1.0000x reference)
"""DPC loss for Trainium2 — transfer-optimal design.

Math (reference):
  p = pred transposed to (M, C), g = gt transposed to (C, M), M=4096, C=256
  lossmat = p @ g                      (M, M)
  loss = -mean(diag(log_softmax(lossmat, axis=1)))
       = mean_r( logsumexp(lossmat[r, :]) - lossmat[r, r] )
  acc  = 100 * mean_r( argmax(lossmat[r, :]) == r )

The device math (one 4096x4096x256 matmul + row reductions, ~9 GFLOP)
takes <1 ms on a NeuronCore; a warm call's wall clock is dominated by
the axon tunnel: ~82 ms dispatch round trip plus ~10-20 ms/MB of
host->device argument transfer. Replicating a tensor to 8 cores costs
8x its bytes and sharded puts are slower than single-device puts
(measured), so every path here uses ONE core and ships minimum bytes;
the other seven cores add transfer cost, not value.

Three paths, fastest applicable wins:

1. Device-resident jax inputs (what setup_inputs() returns under the
   axon platform): run everything in one jit on the device — fp32
   einsum, row logsumexp, diagonal and argmax taken from the SAME
   score matrix (self-consistent), reduce to two scalars. ZERO
   host->device bytes, 8-byte fetch: the call is pure dispatch floor
   (~85 ms). fp32 on the PE matches the reference's own matmul
   rounding; true diag-vs-max margins are O(0.3+), far above it.

2. Numpy inputs: quantize g^T then p^T to int8 (symmetric per-tensor
   scale); g uploads via async device_put overlapping p's
   quantization; total upload 2 MB (a quarter of the fp32 inputs).
   One jit: scores = int8 einsum, int32 accumulation (EXACT —
   |sum| <= 256*127^2 << 2^31), rescale to fp32, row max + stable
   logsumexp, one (2, 4096) fp32 output. The host computes the exact
   diagonal with one einsum while the device runs. Score error from
   quantization is sigma ~0.28 (max ~1.3 observed); loss =
   mean(lse_dev - diag_exact) averages it to ~5e-5 rel (gate 2e-2).
   For the accuracy count, rows with |diag - rowmax| < TAU=2.5 are
   re-decided exactly on the host ((k x 256) @ (256 x 4096) fp32,
   argmax(row) == r — never compare two different fp32 summations of
   the same row); rows outside the band have |true margin| >=
   TAU - max_err > 0, which quantization noise cannot flip.

3. No usable accelerator: exact numpy fp32 fallback.

Jits are compiled at import (persistent neuron compile cache) so the
first kernel() call is already warm.
"""

import numpy as np

B, N, C, H, W = 32, 8, 256, 4, 4
M = B * N * H * W          # 4096
TAU = 2.5                  # ambiguity band; max observed score err ~1.3

_CACHE = {}


def _get_state():
    if "st" in _CACHE:
        return _CACHE["st"]
    st = None
    try:
        import jax
        import jax.numpy as jnp

        try:
            # scrub source paths from HLO metadata so the persistent
            # neuron compile cache hits regardless of the directory
            # this file is imported from
            jax.config.update(
                "jax_hlo_source_file_canonicalization_regex", ".*"
            )
        except Exception:
            pass

        if jax.default_backend() != "cpu":

            def f(qg1, qg2, qp1, qp2, scale):
                # int8 (C/2, M) chunks: column m = row m of p / g.
                # Four ~0.5 MB async uploads pipeline through the
                # tunnel's chunked streaming layer with a much tighter
                # tail than one 2 MB transfer (measured).
                qg = jnp.concatenate([qg1, qg2], axis=0)
                qp = jnp.concatenate([qp1, qp2], axis=0)
                s = jnp.einsum("km,kn->mn", qp, qg,
                               preferred_element_type=jnp.int32)
                s = s.astype(jnp.float32) * scale
                m = jnp.max(s, axis=1)
                lse = m + jnp.log(jnp.sum(jnp.exp(s - m[:, None]), axis=1))
                return jnp.stack([m, lse])  # (2, M) fp32, one 32 KB fetch

            def f2(pred, gt):
                # whole problem on-device from fp32 inputs; diagonal and
                # argmax come from the same score matrix (self-consistent)
                p = jnp.transpose(pred, (0, 1, 3, 4, 2)).reshape(M, C)
                g = jnp.transpose(gt, (2, 0, 1, 3, 4)).reshape(C, M)
                s = jnp.einsum("mk,kn->mn", p, g,
                               preferred_element_type=jnp.float32)
                m = jnp.max(s, axis=1)
                lse = m + jnp.log(jnp.sum(jnp.exp(s - m[:, None]), axis=1))
                diag = jnp.diagonal(s)
                loss = jnp.mean(lse - diag)
                cnt = jnp.sum(
                    (jnp.argmax(s, axis=1) == jnp.arange(M)).astype(jnp.float32)
                )
                return jnp.stack([loss, cnt * (100.0 / M)])

            st = {
                "jax": jax,
                "jf": jax.jit(f),
                "jf2": jax.jit(f2),
                "device_put": jax.device_put,
                "pT": np.empty((C, M), np.float32),
                "gT": np.empty((C, M), np.float32),
                "tmp": np.empty((C, M), np.float32),
                "qp": np.empty((C, M), np.int8),
                "qg": np.empty((C, M), np.int8),
            }
    except Exception:
        st = None
    _CACHE["st"] = st
    return st


def _prep_T(st, dstT, src):
    """(B,N,C,H,W) -> contiguous (C, M) fp32 in dstT; returns int8 scale."""
    np.copyto(dstT, src.transpose(2, 0, 1, 3, 4).reshape(C, M))
    s = np.float32(max(dstT.max(), -float(dstT.min()), 0.0) / 127.0)
    if s == 0.0:
        s = np.float32(1.0)
    return s


def _quant_half(st, dstT, out_i8, scale, lo, hi):
    """Quantize rows lo:hi of dstT into out_i8; returns that slice."""
    t = st["tmp"][lo:hi]
    np.multiply(dstT[lo:hi], np.float32(1.0 / scale), out=t)
    np.rint(t, out=t)              # |t| <= 127.0 by construction
    out_i8[lo:hi] = t              # exact int cast of integral floats
    return out_i8[lo:hi]


def _host_fallback(pT, gT, diag):
    """Reference computation in numpy fp32 (no accelerator needed)."""
    s = pT.T @ gT
    m = s.max(axis=1)
    lse = m + np.log(np.exp(s - m[:, None]).sum(axis=1))
    loss = np.float32(np.mean(lse - diag))
    acc = np.float32(100.0 * (s.argmax(axis=1) == np.arange(M)).sum() / M)
    return loss, acc


def _is_dev_array(st, x):
    try:
        return (
            isinstance(x, st["jax"].Array)
            and x.shape == (B, N, C, H, W)
            and x.dtype == np.float32
            and all(d.platform != "cpu" for d in x.devices())
        )
    except Exception:
        return False


def kernel(pred, gt):
    st = _get_state()

    if st is not None and _is_dev_array(st, pred) and _is_dev_array(st, gt):
        try:
            out = np.asarray(st["jf2"](pred, gt))   # zero-upload fast path
            return np.float32(out[0]), np.float32(out[1])
        except Exception:
            pass

    pred = np.asarray(pred, dtype=np.float32)
    gt = np.asarray(gt, dtype=np.float32)

    if st is None:
        pT = np.ascontiguousarray(pred.transpose(2, 0, 1, 3, 4).reshape(C, M))
        gT = np.ascontiguousarray(gt.transpose(2, 0, 1, 3, 4).reshape(C, M))
        return _host_fallback(pT, gT, np.einsum("cm,cm->m", pT, gT))

    pT, gT = st["pT"], st["gT"]
    half = C // 2
    try:
        put = st["device_put"]
        sg = _prep_T(st, gT, gt)
        g1 = put(_quant_half(st, gT, st["qg"], sg, 0, half))
        g2 = put(_quant_half(st, gT, st["qg"], sg, half, C))
        sp = _prep_T(st, pT, pred)
        p1 = put(_quant_half(st, pT, st["qp"], sp, 0, half))
        p2 = put(_quant_half(st, pT, st["qp"], sp, half, C))
        out = st["jf"](g1, g2, p1, p2, sp * sg)  # async dispatch + compute
        # NOTE: only light single-threaded host work may overlap the
        # device flight — threaded BLAS here delays the tunnel's
        # receive path and costs ~10 ms (measured).
        diag = np.einsum("cm,cm->m", pT, gT)     # exact fp32
        out_h = np.asarray(out)                  # blocks; (2, M)
    except Exception:
        diag = np.einsum("cm,cm->m", pT, gT)
        return _host_fallback(pT, gT, diag)

    m_h, lse_h = out_h[0], out_h[1]
    loss = np.float32(np.mean(lse_h - diag))

    margin = diag - m_h
    ok = margin >= TAU
    amb = np.abs(margin) < TAU
    if amb.any():
        rows = np.nonzero(amb)[0]
        s_rows = pT[:, rows].T @ gT              # exact fp32 rows (k, M)
        ok[rows] = s_rows.argmax(axis=1) == rows
    acc = np.float32(100.0 * ok.sum() / M)
    return loss, acc


def _warmup():
    """Compile + open the tunnel at import so the first call is warm."""
    st = _get_state()
    if st is None:
        return
    try:
        for k in ("pT", "gT", "tmp"):
            st[k].fill(0.0)            # pre-fault host buffers
        for k in ("qp", "qg"):
            st[k].fill(0)
        z = np.zeros((C // 2, M), np.int8)
        zd = st["device_put"](z)
        np.asarray(st["jf"](zd, zd, zd, z, np.float32(1.0)))
    except Exception:
        pass
    try:
        import jax.numpy as jnp

        zd = jnp.zeros((B, N, C, H, W), jnp.float32)
        np.asarray(st["jf2"](zd, zd))
    except Exception:
        pass


_warmup()
